# revision 65
# baseline (speedup 1.0000x reference)
"""BysMamba Trainium2 kernel: 8-core SPMD bass/Tile implementation.

Sharding: core c = (batch b = c//4) x (d_inner shard s = c%4, 128 channels).
Replica groups [[0..3],[4..7]] (one per batch). The fp32 residual stream h
(256 x 2048, replicated within each group) lives in SBUF for the whole
kernel.

Mamba passes (10 total; layers 0 and 9 run BOTH directions fused in one
pass, sharing a single pair of collectives — halves their collective
count and lets the two directions pipeline across engines):
  in_proj with the causal depthwise conv folded into the matmul weights
  (contraction over 256 dims x 4 taps against shifted rhs slices); SiLU on
  ScalarE; x_proj partial + one AllReduce(ndir*48 x 2048 bf16); dt_proj;
  e = Exp(s + b_dt); delta = Ln(1 + e); per state n:
    dA_n = Exp(A_n * delta)   (A_n host-known immediates, on ScalarE)
    dBx_n = (delta*xi) * B_n  (B_n row DMA-replicated from the DRAM
                               collective output)
    h_n = tensor_tensor_scan(dA_n, dBx_n)   -- the hw linear recurrence
                                               (DVE only; walrus rejects
                                               the scan op on GpSimd)
    g_n = h_n * C_n (odd states on GpSimd to offload DVE); identity-matmul
    accumulates sum_n g_n AND d_param*xi into PSUM
  y = psum * SiLU(z); out_proj partial reduced via ReduceScatter +
  AllGather (dodges the AllReduce 1.875x cost); h += out (bf16 residual).
  PSUM->SBUF staging copies run on ScalarE (Copy) to keep DVE free.

Front-end: 3x3 patch conv2d folded on the host into 9 gather tables
(emb @ conv2d_w position slices, center + 0.5 + bias folded); device does
indirect-DMA gathers L-sharded across the group + bf16 AllGather.
Back-end: the FINAL layer's out-reduce is a token-sharded ReduceScatter
with NO AllGather — each core receives (outf + rev(outp)) for exactly its
512 tokens (reverse-direction chunks land in mirrored shard slots, columns
reversed during the ScalarE PSUM->SBUF copy). lm_head on the pre-update h
runs over full L (overlapping the RS, since SPMD cannot bake per-core
token offsets); a 16-matmul delta-lm_head projects the token-local out
delta to logits_d, and the host adds logits[:, my_slice] + logits_d.
"""
import sys
import os

for _p in ("/opt/trn_rl_repo", "/root/.axon_site/_ro/trn_rl_repo"):
    if os.path.isdir(_p) and _p not in sys.path:
        sys.path.insert(0, _p)

import numpy as np
import ml_dtypes

import concourse.bass as bass
import concourse.tile as tile
from concourse import mybir
from concourse.bass_utils import run_bass_kernel_spmd

BF = ml_dtypes.bfloat16
F32 = mybir.dt.float32
BF16 = mybir.dt.bfloat16
I32 = mybir.dt.int32

B = 2
L = 2048
DIM = 256
DIN = 512
DSH = 128
NST = 16
DTR = 16
VOCAB = 474
NM = 10
LPAD = 3
LT = L + LPAD
LSH = 512
NCORES = 8
GROUPS = [[0, 1, 2, 3], [4, 5, 6, 7]]

N_LAYERS = NM          # bring-up override
DEBUG_DUMP_H = False   # adds an "hdump" output with the final residual h
FAKE_COLLECTIVES = False  # timing-only: replace collectives with local DMA
# walrus rejects TensorScalarPtr (scan) on Pool, so scans stay on DVE;
# balance the per-state elementwise multiplies across DVE/GpSimd instead.
POOL_DBX = frozenset()
POOL_GN = frozenset({1, 3, 5, 7, 9, 11, 13})
# bidir passes put both directions' scans on DVE back-to-back, so GpSimd
# can profitably absorb more of their elementwise work
POOL_GN_BI = frozenset({0, 1, 3, 4, 5, 7, 8, 9, 11, 12, 13, 15})
POOL_DBX_BI = frozenset()

_prog_cache = {}


def _split_excess_waits(nc, max_waits=1):
    """walrus here rejects >1 sync-wait per instruction; split the excess
    onto same-engine NoOps placed immediately before."""
    n = 0
    for fn in nc.m.functions:
        for blk in fn.blocks:
            out = []
            changed = False
            for inst in blk.instructions:
                si = inst.sync_info
                waits = list(si.on_wait) if si is not None and si.on_wait else []
                if len(waits) > max_waits:
                    extra = waits[:-max_waits]
                    si.on_wait = waits[-max_waits:]
                    for i in range(0, len(extra), max_waits):
                        out.append(mybir.InstNoOp(
                            name=f"{inst.name}-wsplit-{i}",
                            engine=inst.engine, ins=[], outs=[],
                            sync_info=mybir.SyncInfo(
                                on_wait=extra[i:i + max_waits], on_update=[]),
                        ))
                        n += 1
                    changed = True
                out.append(inst)
            if changed:
                blk.instructions = out
    return n


def _bcast_row_ap(dram_tile_ap, row, width):
    """AP reading one DRAM row replicated across 128 partitions."""
    r = dram_tile_ap[row:row + 1, :]
    return bass.AP(tensor=r.tensor, offset=r.offset, ap=[[0, 128], [1, width]])


def _build_program(a_scales, n_layers, dump_h):
    AOP = mybir.AluOpType
    AF = mybir.ActivationFunctionType

    nc = bass.Bass(num_devices=NCORES)

    def par(name, shape, dt):
        return nc.declare_dram_parameter(name, list(shape), dt, isOutput=False)

    t9 = par("t9", (9 * VOCAB, DIM), F32)
    idxp = par("idxp", (128, 36), I32)
    wconv = par("wconv", (128, NM * 8 * 128), BF16)
    wz = par("wz", (128, NM * 2 * 128), BF16)
    wx = par("wx", (128, NM * 48), BF16)
    wdt = par("wdt", (16, NM * 128), BF16)
    wout = par("wout", (128, NM * 256), BF16)
    lmh = par("lmh", (128, 2 * VOCAB), BF16)
    bdtp = par("bdt", (128, NM), F32)
    cbp = par("cb", (128, NM), F32)
    dprmp = par("dprm", (128, NM), F32)
    b9p = par("b9", (128, 2), F32)
    identb = par("identb", (128, 128), BF16)
    identf = par("identf", (128, 128), F32)

    logits = nc.declare_dram_parameter("logits", [VOCAB, L], BF16, isOutput=True)
    logits_d = nc.declare_dram_parameter("logits_d", [VOCAB, 512], BF16,
                                         isOutput=True)
    hdump = None
    if dump_h:
        hdump = nc.declare_dram_parameter("hdump", [2, 128, LT], BF16, isOutput=True)

    def coll(kind, op, cin, cout, gather_ways=0, flat_gather=False):
        if FAKE_COLLECTIVES:
            if gather_ways and flat_gather:
                rows = cin.shape[0]
                for g in range(gather_ways):
                    nc.sync.dma_start(out=cout[g * rows:(g + 1) * rows],
                                      in_=cin[:])
            elif gather_ways:
                for g in range(gather_ways):
                    nc.sync.dma_start(out=cout[g], in_=cin[:])
            elif kind == "ReduceScatter":
                nc.sync.dma_start(out=cout[:], in_=cin[0])
            else:
                nc.sync.dma_start(out=cout[:], in_=cin[:])
            return
        nc.gpsimd.collective_compute(kind, op, replica_groups=GROUPS,
                                     ins=[cin.opt()], outs=[cout.opt()])

    import contextlib
    with tile.TileContext(nc) as tc, contextlib.ExitStack() as ctx:
        persist = ctx.enter_context(tc.tile_pool(name="persist", bufs=1))
        ps = ctx.enter_context(tc.tile_pool(name="ps", bufs=2, space="PSUM"))
        bc = ctx.enter_context(tc.tile_pool(name="bc", bufs=3))
        wk = ctx.enter_context(tc.tile_pool(name="wk", bufs=2))
        fe = ctx.enter_context(tc.tile_pool(name="fe", bufs=3))
        dram = ctx.enter_context(tc.tile_pool(name="dram", bufs=2, space="DRAM"))

        def ld(param, shape, dt, tag):
            t = persist.tile(list(shape), dt, tag=tag, name=tag)
            nc.sync.dma_start(out=t[:], in_=param[:])
            return t

        wconv_s = ld(wconv, (128, NM * 8 * 128), BF16, "wconv_s")
        wz_s = ld(wz, (128, NM * 2 * 128), BF16, "wz_s")
        wx_s = ld(wx, (128, NM * 48), BF16, "wx_s")
        wdt_s = ld(wdt, (16, NM * 128), BF16, "wdt_s")
        wout_s = ld(wout, (128, NM * 256), BF16, "wout_s")
        lmh_s = ld(lmh, (128, 2 * VOCAB), BF16, "lmh_s")
        bdt_s = ld(bdtp, (128, NM), F32, "bdt_s")
        cb_s = ld(cbp, (128, NM), F32, "cb_s")
        dprm_s = ld(dprmp, (128, NM), F32, "dprm_s")
        b9_s = ld(b9p, (128, 2), F32, "b9_s")
        idb_s = ld(identb, (128, 128), BF16, "idb_s")
        idf_s = ld(identf, (128, 128), F32, "idf_s")
        idx_s = ld(idxp, (128, 36), I32, "idx_s")

        hbf = [persist.tile([128, LT], BF16, tag=f"hbf_{k}", name=f"hbf_{k}") for k in range(2)]
        hrv = [persist.tile([128, LT], BF16, tag=f"hrv_{k}", name=f"hrv_{k}") for k in range(2)]
        for k in range(2):
            nc.vector.memset(hbf[k][:], 0.0)
            nc.vector.memset(hrv[k][:], 0.0)

        xi_t = [persist.tile([128, L], BF16, tag=f"xi_t{s}", name=f"xi_t{s}")
                for s in range(2)]
        sz_t = [persist.tile([128, L], BF16, tag=f"sz_t{s}", name=f"sz_t{s}")
                for s in range(2)]
        u_t = [persist.tile([128, L], BF16, tag=f"u_t{s}", name=f"u_t{s}")
               for s in range(2)]
        dl_t = [persist.tile([128, L], BF16, tag=f"dl_t{s}", name=f"dl_t{s}")
                for s in range(2)]
        dbc16 = [persist.tile([16, L], BF16, tag=f"dbc16_{s}", name=f"dbc16_{s}")
                 for s in range(2)]
        outp = [persist.tile([128, L], BF16, tag=f"outp_{k}", name=f"outp_{k}") for k in range(2)]
        outf = [persist.tile([128, L], BF16, tag=f"outf_{k}", name=f"outf_{k}") for k in range(2)]
        h0loc = [persist.tile([128, LSH], BF16, tag=f"h0loc_{k}", name=f"h0loc_{k}") for k in range(2)]

        # ---- front-end -----------------------------------------------------
        ptt = ps.tile([128, 2048], F32, tag="ps", name="ps")
        for tau in range(4):
            acc = fe.tile([128, DIM], F32, tag="feacc", name="feacc")
            for j in range(9):
                g = fe.tile([128, DIM], F32, tag="feg", name="feg")
                nc.gpsimd.indirect_dma_start(
                    out=g[:], out_offset=None, in_=t9[:],
                    in_offset=bass.IndirectOffsetOnAxis(
                        ap=idx_s[:, tau * 9 + j: tau * 9 + j + 1], axis=0),
                )
                if j == 0:
                    nc.vector.tensor_copy(out=acc[:], in_=g[:])
                else:
                    nc.vector.tensor_tensor(out=acc[:], in0=acc[:], in1=g[:],
                                             op=AOP.add)
            for dh in range(2):
                blk = tau * 2 + dh
                nc.tensor.transpose(
                    out=ptt[:, blk * 128:(blk + 1) * 128],
                    in_=acc[:, dh * 128:(dh + 1) * 128],
                    identity=idf_s[:])
                nc.vector.tensor_scalar(
                    out=h0loc[dh][:, tau * 128:(tau + 1) * 128],
                    in0=ptt[:, blk * 128:(blk + 1) * 128],
                    scalar1=b9_s[:, dh:dh + 1], scalar2=None, op0=AOP.add)

        agi = dram.tile([2, 128, LSH], BF16, tag="agi", name="agi")
        ago = dram.tile([4, 2, 128, LSH], BF16, tag="ago", name="ago")
        for k in range(2):
            nc.sync.dma_start(out=agi[k], in_=h0loc[k][:])
        coll("AllGather", AOP.bypass, agi, ago, gather_ways=4)
        for g in range(4):
            for k in range(2):
                nc.sync.dma_start(
                    out=hbf[k][:, LPAD + g * LSH: LPAD + (g + 1) * LSH],
                    in_=ago[g, k])


        # ---- one (multi-direction) mamba pass ------------------------------
        def mamba_pass(l, dirs, final_rs=False, pre_coll_fn=None):
            """dirs: list of (hb_tiles, out_dst_tiles); bidir passes fuse both
            directions into single collectives and pipelined phases."""
            nd = len(dirs)
            co = l * 8 * 128
            cin = dram.tile([nd * 48, L], BF16, tag="cin", name="cin")
            cout = dram.tile([nd * 48, L], BF16, tag="cout", name="cout")

            # in_proj chunked over L so silu/x_proj/copy pipeline under the
            # matmul tail; z-proj emitted last so it fills the AllReduce
            # window on the PE FIFO
            for s, (hb, _) in enumerate(dirs):
                pxc = ps.tile([128, 2048], F32, tag="ps", name="ps")
                pxp = ps.tile([128, 2048], F32, tag="ps", name="ps")
                dbc_part = wk.tile([48, L], BF16, tag="dbc_part",
                                   name="dbc_part", bufs=1)
                for nt in range(4):
                    for kt in range(2):
                        for j in range(4):
                            lt = wconv_s[:, co + (j * 2 + kt) * 128:
                                         co + (j * 2 + kt) * 128 + 128]
                            nc.tensor.matmul(
                                out=pxc[:, nt * 512:(nt + 1) * 512],
                                lhsT=lt,
                                rhs=hb[kt][:, nt * 512 + j: nt * 512 + j + 512],
                                start=(kt == 0 and j == 0),
                                stop=(kt == 1 and j == 3))
                    nc.scalar.activation(
                        out=xi_t[s][:, nt * 512:(nt + 1) * 512],
                        in_=pxc[:, nt * 512:(nt + 1) * 512], func=AF.Silu,
                        bias=cb_s[:, l:l + 1], scale=1.0)
                    nc.tensor.matmul(
                        out=pxp[:48, nt * 512:(nt + 1) * 512],
                        lhsT=wx_s[:, l * 48:(l + 1) * 48],
                        rhs=xi_t[s][:, nt * 512:(nt + 1) * 512],
                        start=True, stop=True)
                    nc.scalar.activation(
                        out=dbc_part[:, nt * 512:(nt + 1) * 512],
                        in_=pxp[:48, nt * 512:(nt + 1) * 512],
                        func=AF.Copy, scale=1.0)
                nc.sync.dma_start(out=cin[s * 48:(s + 1) * 48], in_=dbc_part[:])

            for s, (hb, _) in enumerate(dirs):
                pz = ps.tile([128, 2048], F32, tag="ps", name="ps")
                for kt in range(2):
                    lt = wz_s[:, (l * 2 + kt) * 128:(l * 2 + kt) * 128 + 128]
                    for nt in range(4):
                        nc.tensor.matmul(
                            out=pz[:, nt * 512:(nt + 1) * 512],
                            lhsT=lt,
                            rhs=hb[kt][:, LPAD + nt * 512: LPAD + nt * 512 + 512],
                            start=(kt == 0), stop=(kt == 1))
                nc.scalar.activation(out=sz_t[s][:], in_=pz[:], func=AF.Silu,
                                     scale=1.0)

            if pre_coll_fn is not None:
                pre_coll_fn()
            rs1 = dram.tile([nd * 12, L], BF16, tag="rs1", name="rs1")
            coll("ReduceScatter", AOP.add, cin, rs1)
            coll("AllGather", AOP.bypass, rs1, cout, gather_ways=4,
                 flat_gather=True)

            for s in range(nd):
                nc.sync.dma_start(out=dbc16[s][:],
                                  in_=cout[s * 48: s * 48 + 16])
                pdt = ps.tile([128, 2048], F32, tag="ps", name="ps")
                for nt in range(4):
                    nc.tensor.matmul(
                        out=pdt[:, nt * 512:(nt + 1) * 512],
                        lhsT=wdt_s[:, l * 128:(l + 1) * 128],
                        rhs=dbc16[s][:, nt * 512:(nt + 1) * 512],
                        start=True, stop=True)
                e_b = wk.tile([128, L], BF16, tag="e_b", name="e_b", bufs=1)
                nc.scalar.activation(out=e_b[:], in_=pdt[:], func=AF.Exp,
                                     bias=bdt_s[:, l:l + 1], scale=1.0)
                nc.scalar.activation(out=dl_t[s][:], in_=e_b[:], func=AF.Ln,
                                     bias=1.0, scale=1.0)
                nc.vector.tensor_tensor(out=u_t[s][:], in0=dl_t[s][:],
                                        in1=xi_t[s][:], op=AOP.mult)

            # state loop: scans mostly on GpSimd, elementwise on DVE,
            # accumulation + d_param*xi fold on PE
            py = [ps.tile([128, 2048], F32, tag="ps", name="ps")
                  for _ in range(nd)]
            for n in range(NST):
                for s in range(nd):
                    bbc = bc.tile([128, L], BF16, tag="bbc", name="bbc")
                    nc.sync.dma_start(
                        out=bbc[:], in_=_bcast_row_ap(cout, s * 48 + 16 + n, L))
                    cbc = bc.tile([128, L], BF16, tag="cbc", name="cbc")
                    nc.sync.dma_start(
                        out=cbc[:], in_=_bcast_row_ap(cout, s * 48 + 32 + n, L))
                    da = wk.tile([128, L], BF16, tag="da", name="da", bufs=3)
                    nc.scalar.activation(out=da[:], in_=dl_t[s][:], func=AF.Exp,
                                         scale=float(a_scales[l][n]))
                    pool_dbx = POOL_DBX_BI if nd == 2 else POOL_DBX
                    pool_gn = POOL_GN_BI if nd == 2 else POOL_GN
                    dbx = wk.tile([128, L], BF16, tag="dbx", name="dbx")
                    dbx_eng = nc.gpsimd if n in pool_dbx else nc.vector
                    dbx_eng.tensor_tensor(out=dbx[:], in0=u_t[s][:],
                                          in1=bbc[:], op=AOP.mult)
                    hn = wk.tile([128, L], BF16, tag="hn", name="hn", bufs=3)
                    nc.vector.tensor_tensor_scan(
                        out=hn[:], data0=da[:], data1=dbx[:], initial=0.0,
                        op0=AOP.mult, op1=AOP.add)
                    gn = wk.tile([128, L], BF16, tag="gn", name="gn")
                    gn_eng = nc.gpsimd if n in pool_gn else nc.vector
                    gn_eng.tensor_tensor(out=gn[:], in0=hn[:], in1=cbc[:],
                                         op=AOP.mult)
                    for nt in range(4):
                        nc.tensor.matmul(
                            out=py[s][:, nt * 512:(nt + 1) * 512],
                            lhsT=idb_s[:],
                            rhs=gn[:, nt * 512:(nt + 1) * 512],
                            start=(n == 0), stop=False)

            # out-reduce as ReduceScatter+AllGather: same result as the old
            # AllReduce but dodges its 1.875x cost; oin's first axis is the
            # 4-way shard axis (flat rows (s*2+mt)*128+p -> shard g, row j)
            if final_rs:
                # final pass: token-sharded ReduceScatter, NO AllGather.
                # shard g holds all nd*256 out-rows for token chunk g; the
                # reverse direction's chunks land in mirrored slots with
                # columns reversed, so shard g is exactly (outf +
                # rev(outp))[:, tokens of group-rank g]
                oin = dram.tile([4, nd * 256, 512], BF16, tag="oin",
                                name="oin")
                rs9 = dram.tile([nd * 256, 512], BF16, tag="rsout",
                                name="rsout")
            else:
                shr = nd * 64
                oin = dram.tile([4, shr, L], BF16, tag="oin", name="oin")
                rsout = dram.tile([shr, L], BF16, tag="rsout", name="rsout")
                agout = dram.tile([4, shr, L], BF16, tag="agout",
                                  name="agout")
            # tail pipelined per L-chunk: each PSUM bank closes with the t1
            # (d_param*xi) matmul, then y2 / out_proj / copy / DMA for that
            # chunk run while later banks are still accumulating
            for s in range(nd):
                t1 = wk.tile([128, L], BF16, tag="e_b", name="t1", bufs=1)
                nc.vector.tensor_scalar(out=t1[:], in0=xi_t[s][:],
                                        scalar1=dprm_s[:, l:l + 1],
                                        scalar2=None, op0=AOP.mult)
                y2 = wk.tile([128, L], BF16, tag="y2", name="y2", bufs=1)
                po = ps.tile([128, 2048], F32, tag="ps", name="ps")
                pob = [wk.tile([128, L], BF16, tag=f"pob{mt}",
                               name=f"pob{mt}", bufs=1) for mt in range(2)]
                for nt in range(4):
                    c0, c1 = nt * 512, (nt + 1) * 512
                    nc.tensor.matmul(out=py[s][:, c0:c1], lhsT=idb_s[:],
                                     rhs=t1[:, c0:c1], start=False, stop=True)
                    nc.vector.tensor_tensor(out=y2[:, c0:c1],
                                            in0=py[s][:, c0:c1],
                                            in1=sz_t[s][:, c0:c1],
                                            op=AOP.mult)
                    for mt in range(2):
                        # mt0/mt1 share PSUM bank nt; the copy-out of mt0
                        # gates mt1's overwrite (write-after-read)
                        nc.tensor.matmul(
                            out=po[:, c0:c1],
                            lhsT=wout_s[:, l * 256 + mt * 128:
                                        l * 256 + mt * 128 + 128],
                            rhs=y2[:, c0:c1], start=True, stop=True)
                        if final_rs and s == 1:
                            # write the reversed-sequence values in-place so
                            # the shard DMA stays dense (a reversed DMA is
                            # ~40x slower in descriptors)
                            rstop = c0 - 1 if c0 > 0 else None
                            nc.scalar.activation(
                                out=pob[mt][:, c1 - 1:rstop:-1],
                                in_=po[:, c0:c1], func=AF.Copy, scale=1.0)
                        else:
                            nc.scalar.activation(out=pob[mt][:, c0:c1],
                                                 in_=po[:, c0:c1],
                                                 func=AF.Copy, scale=1.0)
                        if final_rs:
                            r0 = s * 256 + mt * 128
                            nc.sync.dma_start(
                                out=oin[nt if s == 0 else 3 - nt,
                                        r0:r0 + 128, :],
                                in_=pob[mt][:, c0:c1])
                        elif shr == 128:
                            nc.sync.dma_start(out=oin[s * 2 + mt, :, c0:c1],
                                              in_=pob[mt][:, c0:c1])
                        else:
                            f0 = (s * 2 + mt) * 128
                            nc.sync.dma_start(
                                out=oin[f0 // shr, :, c0:c1],
                                in_=pob[mt][0:64, c0:c1])
                            nc.sync.dma_start(
                                out=oin[f0 // shr + 1, :, c0:c1],
                                in_=pob[mt][64:128, c0:c1])
            if final_rs:
                coll("ReduceScatter", AOP.add, oin, rs9)
                for s, (_, out_dst) in enumerate(dirs):
                    for mt in range(2):
                        r0 = s * 256 + mt * 128
                        nc.sync.dma_start(out=out_dst[mt][:, :512],
                                          in_=rs9[r0:r0 + 128, :])
            else:
                coll("ReduceScatter", AOP.add, oin, rsout)
                coll("AllGather", AOP.bypass, rsout, agout, gather_ways=4)
                for s, (_, out_dst) in enumerate(dirs):
                    for mt in range(2):
                        f0 = (s * 2 + mt) * 128
                        if shr == 128:
                            nc.sync.dma_start(out=out_dst[mt][:],
                                              in_=agout[s * 2 + mt])
                        else:
                            nc.sync.dma_start(out=out_dst[mt][0:64, :],
                                              in_=agout[f0 // shr])
                            nc.sync.dma_start(out=out_dst[mt][64:128, :],
                                              in_=agout[f0 // shr + 1])

        def refresh_hrv():
            for k in range(2):
                nc.vector.tensor_copy(out=hrv[k][:, LPAD:],
                                      in_=hbf[k][:, LT - 1: LPAD - 1: -1])

        # lm_head over full L on the pre-final-update h (host slices per
        # core); emitted inside the final pass's AR1 window where PE idles
        _lmh_a_emitted = [False]

        def emit_lmhead_a():
            _lmh_a_emitted[0] = True
            for mt in range(4):
                m0 = mt * 128
                msz = min(128, VOCAB - m0)
                plh = ps.tile([128, 2048], F32, tag="ps", name="ps")
                lout = wk.tile([128, 2048], BF16, tag="lout", name="lout",
                               bufs=1)
                for nt in range(4):
                    c0, c1 = nt * 512, (nt + 1) * 512
                    for kt in range(2):
                        nc.tensor.matmul(
                            out=plh[:msz, c0:c1],
                            lhsT=lmh_s[:, kt * VOCAB + m0:
                                       kt * VOCAB + m0 + msz],
                            rhs=hbf[kt][:, LPAD + c0: LPAD + c1],
                            start=(kt == 0), stop=(kt == 1))
                    nc.scalar.activation(out=lout[:msz, c0:c1],
                                         in_=plh[:msz, c0:c1],
                                         func=AF.Copy, scale=1.0)
                nc.sync.dma_start(out=logits[m0:m0 + msz, :],
                                  in_=lout[:msz, :])

        # residual updates chunked over L so the next pass's first in_proj
        # chunk starts as soon as its slice of h is current
        for li in range(min(n_layers, NM)):
            bidir = (li == 0 or li == NM - 1)
            if li == NM - 1 and n_layers >= NM:
                # final layer: token-sharded reduce; h never rebuilt.
                # lm_head on the pre-update h runs below (overlapping the
                # RS); the per-core out-delta contribution goes to logits_d
                # and the host adds the two.
                refresh_hrv()
                mamba_pass(li, [(hbf, outf), (hrv, outp)], final_rs=True,
                           pre_coll_fn=emit_lmhead_a)
                continue
            if bidir:
                refresh_hrv()
                mamba_pass(li, [(hbf, outf), (hrv, outp)])
                for nt in range(4):
                    c0, c1 = nt * 512, (nt + 1) * 512
                    for k in range(2):
                        nc.vector.tensor_tensor(
                            out=hbf[k][:, LPAD + c0: LPAD + c1],
                            in0=hbf[k][:, LPAD + c0: LPAD + c1],
                            in1=outf[k][:, c0:c1], op=AOP.add)
                        rstop = L - 1 - c1 if L - 1 - c1 >= 0 else None
                        nc.vector.tensor_tensor(
                            out=hbf[k][:, LPAD + c0: LPAD + c1],
                            in0=hbf[k][:, LPAD + c0: LPAD + c1],
                            in1=outp[k][:, L - 1 - c0: rstop: -1],
                            op=AOP.add)
            else:
                mamba_pass(li, [(hbf, outp)])
                for nt in range(4):
                    c0, c1 = nt * 512, (nt + 1) * 512
                    for k in range(2):
                        nc.vector.tensor_tensor(
                            out=hbf[k][:, LPAD + c0: LPAD + c1],
                            in0=hbf[k][:, LPAD + c0: LPAD + c1],
                            in1=outp[k][:, c0:c1], op=AOP.add)

        if not _lmh_a_emitted[0]:
            emit_lmhead_a()

        # delta lm_head: logits_d = lmh @ (outf + rev(outp))[:, my tokens],
        # fed as 4 k-tiles (outf dims 0-127/128-255, rev-outp same) without
        # materializing the sum
        dtiles = [outf[0], outf[1], outp[0], outp[1]]
        for mt in range(4):
            m0 = mt * 128
            msz = min(128, VOCAB - m0)
            plhd = ps.tile([128, 2048], F32, tag="ps", name="ps")
            for k4 in range(4):
                nc.tensor.matmul(
                    out=plhd[:msz, :512],
                    lhsT=lmh_s[:, (k4 % 2) * VOCAB + m0:
                               (k4 % 2) * VOCAB + m0 + msz],
                    rhs=dtiles[k4][:, :512],
                    start=(k4 == 0), stop=(k4 == 3))
            loutd = wk.tile([128, 512], BF16, tag="loutd", name="loutd",
                            bufs=1)
            nc.scalar.activation(out=loutd[:msz, :], in_=plhd[:msz, :512],
                                 func=AF.Copy, scale=1.0)
            nc.sync.dma_start(out=logits_d[m0:m0 + msz, :],
                              in_=loutd[:msz, :])

        if hdump is not None:
            for k in range(2):
                nc.sync.dma_start(out=hdump[k], in_=hbf[k][:])

    return nc


# --------------------------------------------------------------------------
def _host_prep(inputs):
    f = np.float32
    x = np.asarray(inputs["x"]).astype(np.int64).reshape(B, L, 9)
    emb = np.asarray(inputs["emb"], f)
    c2w = np.asarray(inputs["conv2d_w"], f)
    c2b = np.asarray(inputs["conv2d_b"], f)
    w_in = np.asarray(inputs["w_in"], f)
    conv_w = np.asarray(inputs["conv_w"], f)
    conv_b = np.asarray(inputs["conv_b"], f)
    w_x = np.asarray(inputs["w_x"], f)
    w_dt = np.asarray(inputs["w_dt"], f)
    b_dt = np.asarray(inputs["b_dt"], f)
    a_log = np.asarray(inputs["a_log"], f)
    d_param = np.asarray(inputs["d_param"], f)
    w_out = np.asarray(inputs["w_out"], f)
    lm_head = np.asarray(inputs["lm_head"], f)

    # 9 gather tables: position (i,jj) j=3i+jj; T9[j] = 0.5*emb@c2w[:,:,i,jj].T
    t9 = np.empty((9, VOCAB, DIM), f)
    for j in range(9):
        i, jj = divmod(j, 3)
        t9[j] = 0.5 * (emb @ c2w[:, :, i, jj].T)
    t9[4] += 0.5 * emb
    t9f = np.ascontiguousarray(t9.reshape(9 * VOCAB, DIM))
    b9 = 0.5 * c2b  # (256,)

    a_scales = [[float(-np.exp(a_log[l, 0, n])) for n in range(NST)]
                for l in range(NM)]

    per_core = []
    for c in range(NCORES):
        b, s = divmod(c, 4)
        ds = slice(128 * s, 128 * s + 128)
        dglob = np.arange(128 * s, 128 * s + 128)

        # indices for this core's token slice, flattened into t9f rows
        tok = np.arange(LSH * s, LSH * (s + 1))
        idx = (np.arange(9)[None, :] * VOCAB + x[b][tok]).astype(np.int32)  # (512, 9)
        idxp = np.zeros((128, 36), np.int32)
        for tau in range(4):
            idxp[:, tau * 9:(tau + 1) * 9] = idx[tau * 128:(tau + 1) * 128]

        wconv = np.zeros((128, NM * 8 * 128), BF)
        wzv = np.zeros((128, NM * 2 * 128), BF)
        wxv = np.zeros((128, NM * 48), BF)
        wdtv = np.zeros((16, NM * 128), BF)
        woutv = np.zeros((128, NM * 256), BF)
        for l in range(NM):
            wi = w_in[l][:DIN][ds]          # (128, 256) xi rows
            wzr = w_in[l][DIN:][ds]         # (128, 256) z rows
            cw = conv_w[l][ds]              # (128, 4)
            for j in range(4):
                for kt in range(2):
                    blkc = (l * 8 + j * 2 + kt) * 128
                    # lhsT[kk, d] = cw[d, j] * wi[d, kt*128+kk]
                    wconv[:, blkc:blkc + 128] = (cw[:, j][None, :]
                                                 * wi[:, kt * 128:kt * 128 + 128].T)
            for kt in range(2):
                blkz = (l * 2 + kt) * 128
                wzv[:, blkz:blkz + 128] = wzr[:, kt * 128:kt * 128 + 128].T
            wxv[:, l * 48:(l + 1) * 48] = w_x[l][:, dglob].T  # [d_shard, 48]
            wdtv[:, l * 128:(l + 1) * 128] = w_dt[l][dglob].T  # [16, 128]
            sc = 0.5 if (l == 0 or l == NM - 1) else 1.0
            woutv[:, l * 256:(l + 1) * 256] = sc * w_out[l][:, dglob].T

        lmhv = np.zeros((128, 2 * VOCAB), BF)
        for kt in range(2):
            lmhv[:, kt * VOCAB:(kt + 1) * VOCAB] = lm_head[:, kt * 128:(kt + 1) * 128].T

        per_core.append({
            "t9": t9f,
            "idxp": idxp,
            "wconv": wconv, "wz": wzv, "wx": wxv, "wdt": wdtv, "wout": woutv,
            "lmh": lmhv,
            "bdt": np.ascontiguousarray(b_dt[:, ds].T.astype(f)
                                        if b_dt.ndim == 2 else b_dt),
            "cb": np.ascontiguousarray(conv_b[:, ds].T.astype(f)),
            "dprm": np.ascontiguousarray(d_param[:, ds].T.astype(f)),
            "b9": np.ascontiguousarray(b9.reshape(2, 128).T.astype(f)),
            "identb": np.eye(128, dtype=BF),
            "identf": np.eye(128, dtype=f),
        })
    # bdt shape check: b_dt is (NM, DIN): [:, ds].T -> (128, NM)
    return per_core, a_scales


TRACE = False
LAST_EXEC_NS = None
LAST_RES = None


def _get_prog(a_scales):
    key = ("prog", N_LAYERS, DEBUG_DUMP_H, FAKE_COLLECTIVES)
    if key not in _prog_cache:
        nc = _build_program(a_scales, N_LAYERS, DEBUG_DUMP_H)
        _split_excess_waits(nc)
        _prog_cache[key] = nc
    return _prog_cache[key]


def _run(nc, per_core):
    global LAST_EXEC_NS, LAST_RES
    res = run_bass_kernel_spmd(nc, per_core, core_ids=list(range(NCORES)),
                               trace=TRACE)
    LAST_EXEC_NS = res.exec_time_ns
    LAST_RES = res
    return res


def kernel(**inputs):
    per_core, a_scales = _host_prep(inputs)
    nc = _get_prog(a_scales)
    res = _run(nc, per_core)
    out = np.empty((B, L, VOCAB), np.float32)
    for c in range(NCORES):
        b, s = divmod(c, 4)
        out[b, LSH * s: LSH * (s + 1), :] = (
            res.results[c]["logits"][:, LSH * s: LSH * (s + 1)]
            .astype(np.float32)
            + res.results[c]["logits_d"].astype(np.float32)).T
    if DEBUG_DUMP_H:
        kernel.last_h = [res.results[c].get("hdump") for c in range(NCORES)]
        kernel.last_res = res
    return out



# revision 72
# speedup vs baseline: 1.0023x; 1.0023x over previous
"""BysMamba Trainium2 kernel: 8-core SPMD bass/Tile implementation.

Sharding: core c = (batch b = c//4) x (d_inner shard s = c%4, 128 channels).
Replica groups [[0..3],[4..7]] (one per batch). The fp32 residual stream h
(256 x 2048, replicated within each group) lives in SBUF for the whole
kernel.

Mamba passes (10 total; layers 0 and 9 run BOTH directions fused in one
pass, sharing a single pair of collectives — halves their collective
count and lets the two directions pipeline across engines):
  in_proj with the causal depthwise conv folded into the matmul weights
  (contraction over 256 dims x 4 taps against shifted rhs slices); SiLU on
  ScalarE; x_proj partial + one AllReduce(ndir*48 x 2048 bf16); dt_proj;
  e = Exp(s + b_dt); delta = Ln(1 + e); per state n:
    dA_n = Exp(A_n * delta)   (A_n host-known immediates, on ScalarE)
    dBx_n = (delta*xi) * B_n  (B_n row DMA-replicated from the DRAM
                               collective output)
    h_n = tensor_tensor_scan(dA_n, dBx_n)   -- the hw linear recurrence
                                               (DVE only; walrus rejects
                                               the scan op on GpSimd)
    g_n = h_n * C_n (odd states on GpSimd to offload DVE); identity-matmul
    accumulates sum_n g_n AND d_param*xi into PSUM
  y = psum * SiLU(z); out_proj partial reduced via ReduceScatter +
  AllGather (dodges the AllReduce 1.875x cost); h += out (bf16 residual).
  PSUM->SBUF staging copies run on ScalarE (Copy) to keep DVE free.

Front-end: 3x3 patch conv2d folded on the host into 9 gather tables
(emb @ conv2d_w position slices, center + 0.5 + bias folded); device does
indirect-DMA gathers L-sharded across the group + bf16 AllGather.
Back-end: the FINAL layer's out-reduce is a token-sharded ReduceScatter
with NO AllGather — each core receives (outf + rev(outp)) for exactly its
512 tokens (reverse-direction chunks land in mirrored shard slots, columns
reversed during the ScalarE PSUM->SBUF copy). lm_head on the pre-update h
runs over full L (overlapping the RS, since SPMD cannot bake per-core
token offsets); a 16-matmul delta-lm_head projects the token-local out
delta to logits_d, and the host adds logits[:, my_slice] + logits_d.
"""
import sys
import os

for _p in ("/opt/trn_rl_repo", "/root/.axon_site/_ro/trn_rl_repo"):
    if os.path.isdir(_p) and _p not in sys.path:
        sys.path.insert(0, _p)

import numpy as np
import ml_dtypes

import concourse.bass as bass
import concourse.tile as tile
from concourse import mybir
from concourse.bass_utils import run_bass_kernel_spmd

BF = ml_dtypes.bfloat16
F32 = mybir.dt.float32
BF16 = mybir.dt.bfloat16
I32 = mybir.dt.int32

B = 2
L = 2048
DIM = 256
DIN = 512
DSH = 128
NST = 16
DTR = 16
VOCAB = 474
NM = 10
LPAD = 3
LT = L + LPAD
LSH = 512
NCORES = 8
GROUPS = [[0, 1, 2, 3], [4, 5, 6, 7]]

N_LAYERS = NM          # bring-up override
DEBUG_DUMP_H = False   # adds an "hdump" output with the final residual h
FAKE_COLLECTIVES = False  # timing-only: replace collectives with local DMA
# walrus rejects TensorScalarPtr (scan) on Pool, so scans stay on DVE;
# balance the per-state elementwise multiplies across DVE/GpSimd instead.
POOL_DBX = frozenset()
POOL_GN = frozenset({1, 3, 5, 7, 9, 11, 13})
# bidir passes put both directions' scans on DVE back-to-back, so GpSimd
# can profitably absorb more of their elementwise work
POOL_GN_BI = frozenset({0, 1, 3, 4, 5, 7, 8, 9, 11, 12, 13, 15})
POOL_DBX_BI = frozenset()

_prog_cache = {}


def _split_excess_waits(nc, max_waits=1):
    """walrus here rejects >1 sync-wait per instruction; split the excess
    onto same-engine NoOps placed immediately before."""
    n = 0
    for fn in nc.m.functions:
        for blk in fn.blocks:
            out = []
            changed = False
            for inst in blk.instructions:
                si = inst.sync_info
                waits = list(si.on_wait) if si is not None and si.on_wait else []
                if len(waits) > max_waits:
                    extra = waits[:-max_waits]
                    si.on_wait = waits[-max_waits:]
                    for i in range(0, len(extra), max_waits):
                        out.append(mybir.InstNoOp(
                            name=f"{inst.name}-wsplit-{i}",
                            engine=inst.engine, ins=[], outs=[],
                            sync_info=mybir.SyncInfo(
                                on_wait=extra[i:i + max_waits], on_update=[]),
                        ))
                        n += 1
                    changed = True
                out.append(inst)
            if changed:
                blk.instructions = out
    return n


def _bcast_row_ap(dram_tile_ap, row, width):
    """AP reading one DRAM row replicated across 128 partitions."""
    r = dram_tile_ap[row:row + 1, :]
    return bass.AP(tensor=r.tensor, offset=r.offset, ap=[[0, 128], [1, width]])


def _build_program(a_scales, n_layers, dump_h):
    AOP = mybir.AluOpType
    AF = mybir.ActivationFunctionType

    nc = bass.Bass(num_devices=NCORES)

    def par(name, shape, dt):
        return nc.declare_dram_parameter(name, list(shape), dt, isOutput=False)

    t9 = par("t9", (9 * VOCAB, DIM), F32)
    idxp = par("idxp", (128, 36), I32)
    wconv = par("wconv", (128, NM * 8 * 128), BF16)
    wz = par("wz", (128, NM * 2 * 128), BF16)
    wx = par("wx", (128, NM * 48), BF16)
    wdt = par("wdt", (16, NM * 128), BF16)
    wout = par("wout", (128, NM * 256), BF16)
    lmh = par("lmh", (128, 2 * VOCAB), BF16)
    bdtp = par("bdt", (128, NM), F32)
    cbp = par("cb", (128, NM), F32)
    dprmp = par("dprm", (128, NM), F32)
    b9p = par("b9", (128, 2), F32)
    identb = par("identb", (128, 128), BF16)
    identf = par("identf", (128, 128), F32)

    logits = nc.declare_dram_parameter("logits", [VOCAB, L], BF16, isOutput=True)
    logits_d = nc.declare_dram_parameter("logits_d", [VOCAB, 512], BF16,
                                         isOutput=True)
    hdump = None
    if dump_h:
        hdump = nc.declare_dram_parameter("hdump", [2, 128, LT], BF16, isOutput=True)

    def coll(kind, op, cin, cout, gather_ways=0, flat_gather=False):
        if FAKE_COLLECTIVES:
            if gather_ways and flat_gather:
                rows = cin.shape[0]
                for g in range(gather_ways):
                    nc.sync.dma_start(out=cout[g * rows:(g + 1) * rows],
                                      in_=cin[:])
            elif gather_ways:
                for g in range(gather_ways):
                    nc.sync.dma_start(out=cout[g], in_=cin[:])
            elif kind == "ReduceScatter":
                nc.sync.dma_start(out=cout[:], in_=cin[0])
            else:
                nc.sync.dma_start(out=cout[:], in_=cin[:])
            return
        nc.gpsimd.collective_compute(kind, op, replica_groups=GROUPS,
                                     ins=[cin.opt()], outs=[cout.opt()])

    import contextlib
    with tile.TileContext(nc) as tc, contextlib.ExitStack() as ctx:
        persist = ctx.enter_context(tc.tile_pool(name="persist", bufs=1))
        ps = ctx.enter_context(tc.tile_pool(name="ps", bufs=2, space="PSUM"))
        bc = ctx.enter_context(tc.tile_pool(name="bc", bufs=3))
        wk = ctx.enter_context(tc.tile_pool(name="wk", bufs=2))
        fe = ctx.enter_context(tc.tile_pool(name="fe", bufs=4))
        dram = ctx.enter_context(tc.tile_pool(name="dram", bufs=2, space="DRAM"))

        def ld(param, shape, dt, tag):
            t = persist.tile(list(shape), dt, tag=tag, name=tag)
            nc.sync.dma_start(out=t[:], in_=param[:])
            return t

        wconv_s = ld(wconv, (128, NM * 8 * 128), BF16, "wconv_s")
        wz_s = ld(wz, (128, NM * 2 * 128), BF16, "wz_s")
        wx_s = ld(wx, (128, NM * 48), BF16, "wx_s")
        wdt_s = ld(wdt, (16, NM * 128), BF16, "wdt_s")
        wout_s = ld(wout, (128, NM * 256), BF16, "wout_s")
        lmh_s = ld(lmh, (128, 2 * VOCAB), BF16, "lmh_s")
        bdt_s = ld(bdtp, (128, NM), F32, "bdt_s")
        cb_s = ld(cbp, (128, NM), F32, "cb_s")
        dprm_s = ld(dprmp, (128, NM), F32, "dprm_s")
        b9_s = ld(b9p, (128, 2), F32, "b9_s")
        idb_s = ld(identb, (128, 128), BF16, "idb_s")
        idf_s = ld(identf, (128, 128), F32, "idf_s")
        idx_s = ld(idxp, (128, 36), I32, "idx_s")

        hbf = [persist.tile([128, LT], BF16, tag=f"hbf_{k}", name=f"hbf_{k}") for k in range(2)]
        hrv = [persist.tile([128, LT], BF16, tag=f"hrv_{k}", name=f"hrv_{k}") for k in range(2)]
        for k in range(2):
            nc.vector.memset(hbf[k][:], 0.0)
            nc.vector.memset(hrv[k][:], 0.0)

        xi_t = [persist.tile([128, L], BF16, tag=f"xi_t{s}", name=f"xi_t{s}")
                for s in range(2)]
        sz_t = [persist.tile([128, L], BF16, tag=f"sz_t{s}", name=f"sz_t{s}")
                for s in range(2)]
        u_t = [persist.tile([128, L], BF16, tag=f"u_t{s}", name=f"u_t{s}")
               for s in range(2)]
        dl_t = [persist.tile([128, L], BF16, tag=f"dl_t{s}", name=f"dl_t{s}")
                for s in range(2)]
        dbc16 = [persist.tile([16, L], BF16, tag=f"dbc16_{s}", name=f"dbc16_{s}")
                 for s in range(2)]
        outp = [persist.tile([128, L], BF16, tag=f"outp_{k}", name=f"outp_{k}") for k in range(2)]
        outf = [persist.tile([128, L], BF16, tag=f"outf_{k}", name=f"outf_{k}") for k in range(2)]
        h0loc = [persist.tile([128, LSH], BF16, tag=f"h0loc_{k}", name=f"h0loc_{k}") for k in range(2)]

        # ---- front-end -----------------------------------------------------
        ptt = ps.tile([128, 2048], F32, tag="ps", name="ps")
        for tau in range(4):
            acc = fe.tile([128, DIM], F32, tag="feacc", name="feacc")
            for j in range(9):
                g = fe.tile([128, DIM], F32, tag="feg", name="feg")
                nc.gpsimd.indirect_dma_start(
                    out=g[:], out_offset=None, in_=t9[:],
                    in_offset=bass.IndirectOffsetOnAxis(
                        ap=idx_s[:, tau * 9 + j: tau * 9 + j + 1], axis=0),
                )
                if j == 0:
                    nc.vector.tensor_copy(out=acc[:], in_=g[:])
                else:
                    nc.vector.tensor_tensor(out=acc[:], in0=acc[:], in1=g[:],
                                             op=AOP.add)
            for dh in range(2):
                blk = tau * 2 + dh
                nc.tensor.transpose(
                    out=ptt[:, blk * 128:(blk + 1) * 128],
                    in_=acc[:, dh * 128:(dh + 1) * 128],
                    identity=idf_s[:])
                nc.vector.tensor_scalar(
                    out=h0loc[dh][:, tau * 128:(tau + 1) * 128],
                    in0=ptt[:, blk * 128:(blk + 1) * 128],
                    scalar1=b9_s[:, dh:dh + 1], scalar2=None, op0=AOP.add)

        agi = dram.tile([2, 128, LSH], BF16, tag="agi", name="agi")
        ago = dram.tile([4, 2, 128, LSH], BF16, tag="ago", name="ago")
        for k in range(2):
            nc.sync.dma_start(out=agi[k], in_=h0loc[k][:])
        coll("AllGather", AOP.bypass, agi, ago, gather_ways=4)
        for g in range(4):
            for k in range(2):
                nc.sync.dma_start(
                    out=hbf[k][:, LPAD + g * LSH: LPAD + (g + 1) * LSH],
                    in_=ago[g, k])


        # ---- one (multi-direction) mamba pass ------------------------------
        def mamba_pass(l, dirs, final_rs=False, pre_coll_fn=None):
            """dirs: list of (hb_tiles, out_dst_tiles); bidir passes fuse both
            directions into single collectives and pipelined phases."""
            nd = len(dirs)
            co = l * 8 * 128
            cin = dram.tile([nd * 48, L], BF16, tag="cin", name="cin")
            cout = dram.tile([nd * 48, L], BF16, tag="cout", name="cout")

            # in_proj chunked over L so silu/x_proj/copy pipeline under the
            # matmul tail; z-proj emitted last so it fills the AllReduce
            # window on the PE FIFO
            for s, (hb, _) in enumerate(dirs):
                pxc = ps.tile([128, 2048], F32, tag="ps", name="ps")
                pxp = ps.tile([128, 2048], F32, tag="ps", name="ps")
                dbc_part = wk.tile([48, L], BF16, tag="dbc_part",
                                   name="dbc_part", bufs=1)
                for nt in range(4):
                    for kt in range(2):
                        for j in range(4):
                            lt = wconv_s[:, co + (j * 2 + kt) * 128:
                                         co + (j * 2 + kt) * 128 + 128]
                            nc.tensor.matmul(
                                out=pxc[:, nt * 512:(nt + 1) * 512],
                                lhsT=lt,
                                rhs=hb[kt][:, nt * 512 + j: nt * 512 + j + 512],
                                start=(kt == 0 and j == 0),
                                stop=(kt == 1 and j == 3))
                    nc.scalar.activation(
                        out=xi_t[s][:, nt * 512:(nt + 1) * 512],
                        in_=pxc[:, nt * 512:(nt + 1) * 512], func=AF.Silu,
                        bias=cb_s[:, l:l + 1], scale=1.0)
                    nc.tensor.matmul(
                        out=pxp[:48, nt * 512:(nt + 1) * 512],
                        lhsT=wx_s[:, l * 48:(l + 1) * 48],
                        rhs=xi_t[s][:, nt * 512:(nt + 1) * 512],
                        start=True, stop=True)
                    nc.scalar.activation(
                        out=dbc_part[:, nt * 512:(nt + 1) * 512],
                        in_=pxp[:48, nt * 512:(nt + 1) * 512],
                        func=AF.Copy, scale=1.0)
                nc.sync.dma_start(out=cin[s * 48:(s + 1) * 48], in_=dbc_part[:])

            for s, (hb, _) in enumerate(dirs):
                pz = ps.tile([128, 2048], F32, tag="ps", name="ps")
                for kt in range(2):
                    lt = wz_s[:, (l * 2 + kt) * 128:(l * 2 + kt) * 128 + 128]
                    for nt in range(4):
                        nc.tensor.matmul(
                            out=pz[:, nt * 512:(nt + 1) * 512],
                            lhsT=lt,
                            rhs=hb[kt][:, LPAD + nt * 512: LPAD + nt * 512 + 512],
                            start=(kt == 0), stop=(kt == 1))
                nc.scalar.activation(out=sz_t[s][:], in_=pz[:], func=AF.Silu,
                                     scale=1.0)

            if pre_coll_fn is not None:
                pre_coll_fn()
            rs1 = dram.tile([nd * 12, L], BF16, tag="rs1", name="rs1")
            coll("ReduceScatter", AOP.add, cin, rs1)
            coll("AllGather", AOP.bypass, rs1, cout, gather_ways=4,
                 flat_gather=True)

            for s in range(nd):
                nc.sync.dma_start(out=dbc16[s][:],
                                  in_=cout[s * 48: s * 48 + 16])
                pdt = ps.tile([128, 2048], F32, tag="ps", name="ps")
                for nt in range(4):
                    nc.tensor.matmul(
                        out=pdt[:, nt * 512:(nt + 1) * 512],
                        lhsT=wdt_s[:, l * 128:(l + 1) * 128],
                        rhs=dbc16[s][:, nt * 512:(nt + 1) * 512],
                        start=True, stop=True)
                e_b = wk.tile([128, L], BF16, tag="e_b", name="e_b", bufs=1)
                nc.scalar.activation(out=e_b[:], in_=pdt[:], func=AF.Exp,
                                     bias=bdt_s[:, l:l + 1], scale=1.0)
                nc.scalar.activation(out=dl_t[s][:], in_=e_b[:], func=AF.Ln,
                                     bias=1.0, scale=1.0)
                nc.vector.tensor_tensor(out=u_t[s][:], in0=dl_t[s][:],
                                        in1=xi_t[s][:], op=AOP.mult)

            # state loop: scans mostly on GpSimd, elementwise on DVE,
            # accumulation + d_param*xi fold on PE
            py = [ps.tile([128, 2048], F32, tag="ps", name="ps")
                  for _ in range(nd)]
            for n in range(NST):
                for s in range(nd):
                    bbc = bc.tile([128, L], BF16, tag="bbc", name="bbc")
                    nc.sync.dma_start(
                        out=bbc[:], in_=_bcast_row_ap(cout, s * 48 + 16 + n, L))
                    cbc = bc.tile([128, L], BF16, tag="cbc", name="cbc")
                    nc.sync.dma_start(
                        out=cbc[:], in_=_bcast_row_ap(cout, s * 48 + 32 + n, L))
                    da = wk.tile([128, L], BF16, tag="da", name="da", bufs=3)
                    nc.scalar.activation(out=da[:], in_=dl_t[s][:], func=AF.Exp,
                                         scale=float(a_scales[l][n]))
                    pool_dbx = POOL_DBX_BI if nd == 2 else POOL_DBX
                    pool_gn = POOL_GN_BI if nd == 2 else POOL_GN
                    dbx = wk.tile([128, L], BF16, tag="dbx", name="dbx")
                    dbx_eng = nc.gpsimd if n in pool_dbx else nc.vector
                    dbx_eng.tensor_tensor(out=dbx[:], in0=u_t[s][:],
                                          in1=bbc[:], op=AOP.mult)
                    hn = wk.tile([128, L], BF16, tag="hn", name="hn", bufs=3)
                    nc.vector.tensor_tensor_scan(
                        out=hn[:], data0=da[:], data1=dbx[:], initial=0.0,
                        op0=AOP.mult, op1=AOP.add)
                    gn = wk.tile([128, L], BF16, tag="gn", name="gn")
                    gn_eng = nc.gpsimd if n in pool_gn else nc.vector
                    gn_eng.tensor_tensor(out=gn[:], in0=hn[:], in1=cbc[:],
                                         op=AOP.mult)
                    for nt in range(4):
                        nc.tensor.matmul(
                            out=py[s][:, nt * 512:(nt + 1) * 512],
                            lhsT=idb_s[:],
                            rhs=gn[:, nt * 512:(nt + 1) * 512],
                            start=(n == 0), stop=False)

            # out-reduce as ReduceScatter+AllGather: same result as the old
            # AllReduce but dodges its 1.875x cost; oin's first axis is the
            # 4-way shard axis (flat rows (s*2+mt)*128+p -> shard g, row j)
            if final_rs:
                # final pass: token-sharded ReduceScatter, NO AllGather.
                # shard g holds all nd*256 out-rows for token chunk g; the
                # reverse direction's chunks land in mirrored slots with
                # columns reversed, so shard g is exactly (outf +
                # rev(outp))[:, tokens of group-rank g]
                oin = dram.tile([4, nd * 256, 512], BF16, tag="oin",
                                name="oin")
                rs9 = dram.tile([nd * 256, 512], BF16, tag="rsout",
                                name="rsout")
            else:
                shr = nd * 64
                oin = dram.tile([4, shr, L], BF16, tag="oin", name="oin")
                rsout = dram.tile([shr, L], BF16, tag="rsout", name="rsout")
                agout = dram.tile([4, shr, L], BF16, tag="agout",
                                  name="agout")
            # tail pipelined per L-chunk: each PSUM bank closes with the t1
            # (d_param*xi) matmul, then y2 / out_proj / copy / DMA for that
            # chunk run while later banks are still accumulating
            for s in range(nd):
                t1 = wk.tile([128, L], BF16, tag="e_b", name="t1", bufs=1)
                nc.vector.tensor_scalar(out=t1[:], in0=xi_t[s][:],
                                        scalar1=dprm_s[:, l:l + 1],
                                        scalar2=None, op0=AOP.mult)
                y2 = wk.tile([128, L], BF16, tag="y2", name="y2", bufs=1)
                po = ps.tile([128, 2048], F32, tag="ps", name="ps")
                pob = [wk.tile([128, L], BF16, tag=f"pob{mt}",
                               name=f"pob{mt}", bufs=1) for mt in range(2)]
                for nt in range(4):
                    c0, c1 = nt * 512, (nt + 1) * 512
                    nc.tensor.matmul(out=py[s][:, c0:c1], lhsT=idb_s[:],
                                     rhs=t1[:, c0:c1], start=False, stop=True)
                    nc.vector.tensor_tensor(out=y2[:, c0:c1],
                                            in0=py[s][:, c0:c1],
                                            in1=sz_t[s][:, c0:c1],
                                            op=AOP.mult)
                    for mt in range(2):
                        # mt0/mt1 share PSUM bank nt; the copy-out of mt0
                        # gates mt1's overwrite (write-after-read)
                        nc.tensor.matmul(
                            out=po[:, c0:c1],
                            lhsT=wout_s[:, l * 256 + mt * 128:
                                        l * 256 + mt * 128 + 128],
                            rhs=y2[:, c0:c1], start=True, stop=True)
                        if final_rs and s == 1:
                            # write the reversed-sequence values in-place so
                            # the shard DMA stays dense (a reversed DMA is
                            # ~40x slower in descriptors)
                            rstop = c0 - 1 if c0 > 0 else None
                            nc.scalar.activation(
                                out=pob[mt][:, c1 - 1:rstop:-1],
                                in_=po[:, c0:c1], func=AF.Copy, scale=1.0)
                        else:
                            nc.scalar.activation(out=pob[mt][:, c0:c1],
                                                 in_=po[:, c0:c1],
                                                 func=AF.Copy, scale=1.0)
                        if final_rs:
                            r0 = s * 256 + mt * 128
                            nc.sync.dma_start(
                                out=oin[nt if s == 0 else 3 - nt,
                                        r0:r0 + 128, :],
                                in_=pob[mt][:, c0:c1])
                        elif shr == 128:
                            nc.sync.dma_start(out=oin[s * 2 + mt, :, c0:c1],
                                              in_=pob[mt][:, c0:c1])
                        else:
                            f0 = (s * 2 + mt) * 128
                            nc.sync.dma_start(
                                out=oin[f0 // shr, :, c0:c1],
                                in_=pob[mt][0:64, c0:c1])
                            nc.sync.dma_start(
                                out=oin[f0 // shr + 1, :, c0:c1],
                                in_=pob[mt][64:128, c0:c1])
            if final_rs:
                coll("ReduceScatter", AOP.add, oin, rs9)
                for s, (_, out_dst) in enumerate(dirs):
                    for mt in range(2):
                        r0 = s * 256 + mt * 128
                        nc.sync.dma_start(out=out_dst[mt][:, :512],
                                          in_=rs9[r0:r0 + 128, :])
            else:
                coll("ReduceScatter", AOP.add, oin, rsout)
                coll("AllGather", AOP.bypass, rsout, agout, gather_ways=4)
                for s, (_, out_dst) in enumerate(dirs):
                    for mt in range(2):
                        f0 = (s * 2 + mt) * 128
                        if shr == 128:
                            nc.sync.dma_start(out=out_dst[mt][:],
                                              in_=agout[s * 2 + mt])
                        else:
                            nc.sync.dma_start(out=out_dst[mt][0:64, :],
                                              in_=agout[f0 // shr])
                            nc.sync.dma_start(out=out_dst[mt][64:128, :],
                                              in_=agout[f0 // shr + 1])

        def refresh_hrv():
            for k in range(2):
                nc.vector.tensor_copy(out=hrv[k][:, LPAD:],
                                      in_=hbf[k][:, LT - 1: LPAD - 1: -1])

        # lm_head over full L on the pre-final-update h (host slices per
        # core); emitted inside the final pass's AR1 window where PE idles
        _lmh_a_emitted = [False]

        def emit_lmhead_a():
            _lmh_a_emitted[0] = True
            for mt in range(4):
                m0 = mt * 128
                msz = min(128, VOCAB - m0)
                plh = ps.tile([128, 2048], F32, tag="ps", name="ps")
                lout = wk.tile([128, 2048], BF16, tag="lout", name="lout",
                               bufs=1)
                for nt in range(4):
                    c0, c1 = nt * 512, (nt + 1) * 512
                    for kt in range(2):
                        nc.tensor.matmul(
                            out=plh[:msz, c0:c1],
                            lhsT=lmh_s[:, kt * VOCAB + m0:
                                       kt * VOCAB + m0 + msz],
                            rhs=hbf[kt][:, LPAD + c0: LPAD + c1],
                            start=(kt == 0), stop=(kt == 1))
                    nc.scalar.activation(out=lout[:msz, c0:c1],
                                         in_=plh[:msz, c0:c1],
                                         func=AF.Copy, scale=1.0)
                nc.sync.dma_start(out=logits[m0:m0 + msz, :],
                                  in_=lout[:msz, :])

        # residual updates chunked over L so the next pass's first in_proj
        # chunk starts as soon as its slice of h is current
        for li in range(min(n_layers, NM)):
            bidir = (li == 0 or li == NM - 1)
            if li == NM - 1 and n_layers >= NM:
                # final layer: token-sharded reduce; h never rebuilt.
                # lm_head on the pre-update h runs below (overlapping the
                # RS); the per-core out-delta contribution goes to logits_d
                # and the host adds the two.
                refresh_hrv()
                mamba_pass(li, [(hbf, outf), (hrv, outp)], final_rs=True,
                           pre_coll_fn=emit_lmhead_a)
                continue
            if bidir:
                refresh_hrv()
                mamba_pass(li, [(hbf, outf), (hrv, outp)])
                for nt in range(4):
                    c0, c1 = nt * 512, (nt + 1) * 512
                    for k in range(2):
                        nc.vector.tensor_tensor(
                            out=hbf[k][:, LPAD + c0: LPAD + c1],
                            in0=hbf[k][:, LPAD + c0: LPAD + c1],
                            in1=outf[k][:, c0:c1], op=AOP.add)
                        rstop = L - 1 - c1 if L - 1 - c1 >= 0 else None
                        nc.vector.tensor_tensor(
                            out=hbf[k][:, LPAD + c0: LPAD + c1],
                            in0=hbf[k][:, LPAD + c0: LPAD + c1],
                            in1=outp[k][:, L - 1 - c0: rstop: -1],
                            op=AOP.add)
            else:
                mamba_pass(li, [(hbf, outp)])
                for nt in range(4):
                    c0, c1 = nt * 512, (nt + 1) * 512
                    for k in range(2):
                        nc.vector.tensor_tensor(
                            out=hbf[k][:, LPAD + c0: LPAD + c1],
                            in0=hbf[k][:, LPAD + c0: LPAD + c1],
                            in1=outp[k][:, c0:c1], op=AOP.add)

        if not _lmh_a_emitted[0]:
            emit_lmhead_a()

        # delta lm_head: logits_d = lmh @ (outf + rev(outp))[:, my tokens],
        # fed as 4 k-tiles (outf dims 0-127/128-255, rev-outp same) without
        # materializing the sum
        dtiles = [outf[0], outf[1], outp[0], outp[1]]
        for mt in range(4):
            m0 = mt * 128
            msz = min(128, VOCAB - m0)
            plhd = ps.tile([128, 2048], F32, tag="ps", name="ps")
            for k4 in range(4):
                nc.tensor.matmul(
                    out=plhd[:msz, :512],
                    lhsT=lmh_s[:, (k4 % 2) * VOCAB + m0:
                               (k4 % 2) * VOCAB + m0 + msz],
                    rhs=dtiles[k4][:, :512],
                    start=(k4 == 0), stop=(k4 == 3))
            loutd = wk.tile([128, 512], BF16, tag="loutd", name="loutd",
                            bufs=1)
            nc.scalar.activation(out=loutd[:msz, :], in_=plhd[:msz, :512],
                                 func=AF.Copy, scale=1.0)
            nc.sync.dma_start(out=logits_d[m0:m0 + msz, :],
                              in_=loutd[:msz, :])

        if hdump is not None:
            for k in range(2):
                nc.sync.dma_start(out=hdump[k], in_=hbf[k][:])

    return nc


# --------------------------------------------------------------------------
def _host_prep(inputs):
    f = np.float32
    x = np.asarray(inputs["x"]).astype(np.int64).reshape(B, L, 9)
    emb = np.asarray(inputs["emb"], f)
    c2w = np.asarray(inputs["conv2d_w"], f)
    c2b = np.asarray(inputs["conv2d_b"], f)
    w_in = np.asarray(inputs["w_in"], f)
    conv_w = np.asarray(inputs["conv_w"], f)
    conv_b = np.asarray(inputs["conv_b"], f)
    w_x = np.asarray(inputs["w_x"], f)
    w_dt = np.asarray(inputs["w_dt"], f)
    b_dt = np.asarray(inputs["b_dt"], f)
    a_log = np.asarray(inputs["a_log"], f)
    d_param = np.asarray(inputs["d_param"], f)
    w_out = np.asarray(inputs["w_out"], f)
    lm_head = np.asarray(inputs["lm_head"], f)

    # 9 gather tables: position (i,jj) j=3i+jj; T9[j] = 0.5*emb@c2w[:,:,i,jj].T
    t9 = np.empty((9, VOCAB, DIM), f)
    for j in range(9):
        i, jj = divmod(j, 3)
        t9[j] = 0.5 * (emb @ c2w[:, :, i, jj].T)
    t9[4] += 0.5 * emb
    t9f = np.ascontiguousarray(t9.reshape(9 * VOCAB, DIM))
    b9 = 0.5 * c2b  # (256,)

    a_scales = [[float(-np.exp(a_log[l, 0, n])) for n in range(NST)]
                for l in range(NM)]

    per_core = []
    for c in range(NCORES):
        b, s = divmod(c, 4)
        ds = slice(128 * s, 128 * s + 128)
        dglob = np.arange(128 * s, 128 * s + 128)

        # indices for this core's token slice, flattened into t9f rows
        tok = np.arange(LSH * s, LSH * (s + 1))
        idx = (np.arange(9)[None, :] * VOCAB + x[b][tok]).astype(np.int32)  # (512, 9)
        idxp = np.zeros((128, 36), np.int32)
        for tau in range(4):
            idxp[:, tau * 9:(tau + 1) * 9] = idx[tau * 128:(tau + 1) * 128]

        wconv = np.zeros((128, NM * 8 * 128), BF)
        wzv = np.zeros((128, NM * 2 * 128), BF)
        wxv = np.zeros((128, NM * 48), BF)
        wdtv = np.zeros((16, NM * 128), BF)
        woutv = np.zeros((128, NM * 256), BF)
        for l in range(NM):
            wi = w_in[l][:DIN][ds]          # (128, 256) xi rows
            wzr = w_in[l][DIN:][ds]         # (128, 256) z rows
            cw = conv_w[l][ds]              # (128, 4)
            for j in range(4):
                for kt in range(2):
                    blkc = (l * 8 + j * 2 + kt) * 128
                    # lhsT[kk, d] = cw[d, j] * wi[d, kt*128+kk]
                    wconv[:, blkc:blkc + 128] = (cw[:, j][None, :]
                                                 * wi[:, kt * 128:kt * 128 + 128].T)
            for kt in range(2):
                blkz = (l * 2 + kt) * 128
                wzv[:, blkz:blkz + 128] = wzr[:, kt * 128:kt * 128 + 128].T
            wxv[:, l * 48:(l + 1) * 48] = w_x[l][:, dglob].T  # [d_shard, 48]
            wdtv[:, l * 128:(l + 1) * 128] = w_dt[l][dglob].T  # [16, 128]
            sc = 0.5 if (l == 0 or l == NM - 1) else 1.0
            woutv[:, l * 256:(l + 1) * 256] = sc * w_out[l][:, dglob].T

        lmhv = np.zeros((128, 2 * VOCAB), BF)
        for kt in range(2):
            lmhv[:, kt * VOCAB:(kt + 1) * VOCAB] = lm_head[:, kt * 128:(kt + 1) * 128].T

        per_core.append({
            "t9": t9f,
            "idxp": idxp,
            "wconv": wconv, "wz": wzv, "wx": wxv, "wdt": wdtv, "wout": woutv,
            "lmh": lmhv,
            "bdt": np.ascontiguousarray(b_dt[:, ds].T.astype(f)
                                        if b_dt.ndim == 2 else b_dt),
            "cb": np.ascontiguousarray(conv_b[:, ds].T.astype(f)),
            "dprm": np.ascontiguousarray(d_param[:, ds].T.astype(f)),
            "b9": np.ascontiguousarray(b9.reshape(2, 128).T.astype(f)),
            "identb": np.eye(128, dtype=BF),
            "identf": np.eye(128, dtype=f),
        })
    # bdt shape check: b_dt is (NM, DIN): [:, ds].T -> (128, NM)
    return per_core, a_scales


TRACE = False
LAST_EXEC_NS = None
LAST_RES = None


def _get_prog(a_scales):
    key = ("prog", N_LAYERS, DEBUG_DUMP_H, FAKE_COLLECTIVES)
    if key not in _prog_cache:
        nc = _build_program(a_scales, N_LAYERS, DEBUG_DUMP_H)
        _split_excess_waits(nc)
        _prog_cache[key] = nc
    return _prog_cache[key]


def _run(nc, per_core):
    global LAST_EXEC_NS, LAST_RES
    res = run_bass_kernel_spmd(nc, per_core, core_ids=list(range(NCORES)),
                               trace=TRACE)
    LAST_EXEC_NS = res.exec_time_ns
    LAST_RES = res
    return res


def kernel(**inputs):
    per_core, a_scales = _host_prep(inputs)
    nc = _get_prog(a_scales)
    res = _run(nc, per_core)
    out = np.empty((B, L, VOCAB), np.float32)
    for c in range(NCORES):
        b, s = divmod(c, 4)
        out[b, LSH * s: LSH * (s + 1), :] = (
            res.results[c]["logits"][:, LSH * s: LSH * (s + 1)]
            .astype(np.float32)
            + res.results[c]["logits_d"].astype(np.float32)).T
    if DEBUG_DUMP_H:
        kernel.last_h = [res.results[c].get("hdump") for c in range(NCORES)]
        kernel.last_res = res
    return out

